# revision 64
# baseline (speedup 1.0000x reference)
"""Trainium2 Bass kernel for multi-head attention (B=8, N=1024, C=1024, H=16).

Sharding: pure data parallel — one batch element per NeuronCore (8 cores),
no collectives. Host pre-transposes/casts weights and activations to bf16;
all matmuls run bf16 with fp32 PSUM accumulation.

Per-core layout strategy (everything derived so softmax needs no transposes):
  - qT,kT computed as [d, n] (head dim on partitions)  -> scores come out
    transposed: S^T[nk, nq] with softmax axis on PARTITIONS.
  - exp(S^T) via ScalarE (scale=1/sqrt(D) folded in, no max-subtraction:
    |scores| <= ~4 for this problem so fp32 exp is safe).
  - rowsum obtained free by appending a ones-column to V (lhsT [nk, 65]);
    PV matmul yields [O'^T ; rowsum] in one accumulation group.
  - normalization fully off the TensorEngine: reciprocal_approx_fast on the
    PSUM rowsum row, gpsimd partition_broadcast, then elementwise multiply.
  - biases: when nonzero, folded in as K=1 accumulation matmuls (ones row in
    xT / bias row appended to the transposed weights); skipped when zero.
"""

import sys

import numpy as np

if "/opt/trn_rl_repo" not in sys.path:
    sys.path.insert(0, "/opt/trn_rl_repo")

import ml_dtypes

BF16 = ml_dtypes.bfloat16

C = 1024          # model dim
N = 1024          # sequence length
H = 16            # heads
D = 64            # head dim
B = 8             # batch == number of cores
KT = C // 128     # 8 contraction tiles
NT = N // 128     # 8 sequence tiles
SCALE = float(D) ** -0.5

_CACHE = {}
LAST_RESULTS = None


def _build_graph(nc, tc, bass, mybir, has_bias):
    from contextlib import ExitStack

    f32 = mybir.dt.float32
    bf16 = mybir.dt.bfloat16
    Exp = mybir.ActivationFunctionType.Exp

    xT_d = nc.dram_tensor("xT", [C + 1, N], bf16, kind="ExternalInput").ap()
    wq_d = nc.dram_tensor("wqkvT", [C + 1, 3 * C], bf16, kind="ExternalInput").ap()
    wp_d = nc.dram_tensor("wprojT", [C + 1, C], bf16, kind="ExternalInput").ap()
    out_d = nc.dram_tensor("out", [N, C], f32, kind="ExternalOutput").ap()

    with ExitStack() as ctx:
        persist = ctx.enter_context(tc.tile_pool(name="persist", bufs=1))
        qkp = ctx.enter_context(tc.tile_pool(name="qkp", bufs=4))
        expp = ctx.enter_context(tc.tile_pool(name="expp", bufs=16))
        small = ctx.enter_context(tc.tile_pool(name="small", bufs=3))
        outp = ctx.enter_context(tc.tile_pool(name="outp", bufs=2))
        # PSUM budget = 8 banks: "mm" 2x[128,512] (2) + "s" 2x[128,1024] (4)
        # + "o" 2x[65,512] (2).
        pmm = ctx.enter_context(tc.tile_pool(name="pmm", bufs=2, space="PSUM"))
        pss = ctx.enter_context(tc.tile_pool(name="pss", bufs=2, space="PSUM"))
        po = ctx.enter_context(tc.tile_pool(name="po", bufs=2, space="PSUM"))
        drp = ctx.enter_context(tc.tile_pool(name="drp", bufs=2, space="DRAM"))

        # ---- persistent SBUF tensors ----
        xt = [persist.tile([128, N], bf16, tag=f"xt{i}", name=f"xt{i}") for i in range(KT)]
        wq = [persist.tile([128, 3 * C], bf16, tag=f"wq{i}", name=f"wq{i}") for i in range(KT)]
        wp = [persist.tile([128, C], bf16, tag=f"wp{i}", name=f"wp{i}") for i in range(KT)]
        vv = [persist.tile([128, H * 65], bf16, tag=f"vv{i}", name=f"vv{i}") for i in range(NT)]
        ot = [persist.tile([128, N], bf16, tag=f"ot{i}", name=f"ot{i}") for i in range(KT)]
        if has_bias:
            xones = persist.tile([1, N], bf16, tag="xones", name="xones")
            wqb = persist.tile([1, 3 * C], bf16, tag="wqb", name="wqb")
            wpb = persist.tile([1, C], bf16, tag="wpb", name="wpb")

        # ---- input DMAs: striped by contraction tile across BOTH DMA queues
        # (xt[i]+wq[i] land together, in kt order) so stage-1 accumulation
        # paces with arrivals; wp (needed last) rides behind on the SWDGE
        # queue.
        for i in range(KT):
            eng = nc.sync if i % 2 == 0 else nc.gpsimd
            eng.dma_start(xt[i][:], xT_d[i * 128:(i + 1) * 128, :])
            eng.dma_start(wq[i][:], wq_d[i * 128:(i + 1) * 128, :])
        for i in range(KT):
            nc.gpsimd.dma_start(wp[i][:], wp_d[i * 128:(i + 1) * 128, :])
        if has_bias:
            nc.sync.dma_start(xones[:], xT_d[C:C + 1, :])
            nc.sync.dma_start(wqb[:], wq_d[C:C + 1, :])
            nc.sync.dma_start(wpb[:], wp_d[C:C + 1, :])

        # preload the Exp activation table during the DMA phase so the first
        # real exp doesn't stall the score pipeline ~2.7us.
        warm = small.tile([1, 16], f32, tag="warm", name="warm")
        nc.gpsimd.memset(warm[:], 0.0)
        nc.scalar.activation(warm[:], warm[:], Exp, scale=1.0)

        qk = {}  # o-tile index (0..7 = q, 8..15 = k) -> sbuf tile

        def qk_tile(j, pools=None):
            """Orientation A: qkT[o_tile j, n] = w_qkvT[:, o].T @ xT  (o on partitions)."""
            pools = pools or (pmm, pmm)
            t = qkp.tile([128, N], bf16, tag="qk", name=f"qk{j}")
            for half in range(2):
                sl = bass.ts(half, 512)
                pool = pools[half]
                ps = pool.tile([128, 512], f32,
                               tag="mm" if pool is pmm else "s",
                               name=f"ps_qk{j}_{half}")
                for kt in range(KT):
                    nc.tensor.matmul(
                        ps[:], wq[kt][:, j * 128:(j + 1) * 128], xt[kt][:, sl],
                        start=(kt == 0), stop=(kt == KT - 1 and not has_bias))
                if has_bias:
                    nc.tensor.matmul(
                        ps[:], wqb[:, j * 128:(j + 1) * 128], xones[:, sl],
                        start=False, stop=True)
                nc.vector.tensor_copy(t[:, sl], ps[:])
            qk[j] = t

        def v_tile(nt, pools=None):
            """Orientation B: v[n_tile, o] = xT[:, n].T @ w_qkvT[:, 2C:]  (n on partitions).
            Stored with stride-65 head blocks; col 64 of each block = ones (rowsum trick)."""
            pools = pools or (pmm, pmm)
            dst = vv[nt][:].rearrange("p (h w) -> p h w", w=65)
            for half in range(2):
                sl = bass.ds(2 * C + half * 512, 512)
                pool = pools[half]
                ps = pool.tile([128, 512], f32,
                               tag="mm" if pool is pmm else "s",
                               name=f"ps_v{nt}_{half}")
                for kt in range(KT):
                    nc.tensor.matmul(
                        ps[:], xt[kt][:, nt * 128:(nt + 1) * 128], wq[kt][:, sl],
                        start=(kt == 0), stop=(kt == KT - 1 and not has_bias))
                if has_bias:
                    nc.tensor.matmul(
                        ps[:], xones[:, nt * 128:(nt + 1) * 128], wqb[:, sl],
                        start=False, stop=True)
                nc.vector.tensor_copy(
                    dst[:, half * 8:(half + 1) * 8, 0:64],
                    ps[:].rearrange("p (h w) -> p h w", w=64))
            nc.gpsimd.memset(dst[:, :, 64:65], 1.0)

        def scores_j(h0, h1, j):
            """One nk-tile of pair scores. Two per-head [128, 1024] PSUM tiles
            (so exp(j) on one overlaps scores(j+1) on the other — a single
            slot ping-pongs ACT against the PE); the 4 K=64 matmuls alternate
            row groups 0/64 so the PE runs the two heads concurrently. exp_A
            is emitted after the 3rd matmul so ACT starts half a tile early."""
            qs0 = qk[h0 // 2][0:64, :]
            ks0 = qk[8 + h0 // 2][0:64, :]
            qs1 = qk[h1 // 2][64:128, :]
            ks1 = qk[8 + h1 // 2][64:128, :]
            jsl = slice(j * 128, (j + 1) * 128)
            psA = pss.tile([128, N], f32, tag="s", name=f"ps_s{h0}_{j}")
            psB = pss.tile([128, N], f32, tag="s", name=f"ps_s{h1}_{j}")
            nc.tensor.matmul(psA[:, 0:512], ks0[:, jsl], qs0[:, 0:512],
                             start=True, stop=True)
            nc.tensor.matmul(psB[:, 0:512], ks1[:, jsl], qs1[:, 0:512],
                             start=True, stop=True)
            nc.tensor.matmul(psA[:, 512:1024], ks0[:, jsl], qs0[:, 512:1024],
                             start=True, stop=True)
            eA = expp.tile([128, N], bf16, tag="es", name=f"es{h0}_{j}")
            nc.scalar.activation(eA[:], psA[:], Exp, scale=SCALE)
            nc.tensor.matmul(psB[:, 512:1024], ks1[:, jsl], qs1[:, 512:1024],
                             start=True, stop=True)
            eB = expp.tile([128, N], bf16, tag="es", name=f"es{h1}_{j}")
            nc.scalar.activation(eB[:], psB[:], Exp, scale=SCALE)
            return eA, eB

        def pv_step(h, psos, j, e):
            """One nk-tile of [O'^T ; rowsum] accumulation (both nq halves)."""
            for half in range(2):
                esl = bass.ts(half, 512)
                nc.tensor.matmul(
                    psos[half][:], vv[j][:, h * 65:(h + 1) * 65], e[:, esl],
                    start=(j == 0), stop=(j == NT - 1))

        def norm(h, psos, halves=(0, 1)):
            """Normalize O'^T by its rowsum into ot. Drain PSUM to SBUF first
            (frees the po slots; the custom-DVE reciprocal also misreads PSUM
            on HW); partition-broadcast via a DRAM bounce on the SWDGE queue
            (SBUF APs cannot have step-0 partition dims)."""
            off = (h % 2) * 64
            for half in halves:
                sl = bass.ts(half, 512)
                pso = psos[half]
                o_sb = small.tile([64, 512], bf16, tag="osb2", name=f"o_sb{h}_{half}")
                nc.vector.tensor_copy(o_sb[:], pso[0:64, :])
                srow = small.tile([1, 512], f32, tag="srow", name=f"srow{h}_{half}")
                nc.vector.tensor_copy(srow[:], pso[64:65, :])
                r1 = small.tile([1, 512], f32, tag="rc", name=f"rc{h}_{half}")
                nc.vector.reciprocal_approx_fast(out=r1[:], in_=srow[:])
                r1b = small.tile([1, 512], bf16, tag="rcb", name=f"rcb{h}_{half}")
                nc.vector.tensor_copy(r1b[:], r1[:])
                scr = drp.tile([1, 512], bf16, tag="scr", name=f"scr{h}_{half}")
                nc.gpsimd.dma_start(scr[:], r1b[:])
                s = scr[:]
                src_b = bass.AP(tensor=s.tensor, offset=s.offset,
                                ap=[[0, 64]] + list(s.ap[1:]))
                rbc = small.tile([64, 512], bf16, tag="rbc", name=f"rbc{h}_{half}")
                nc.gpsimd.dma_start(rbc[:], src_b)
                nc.vector.tensor_mul(ot[h // 2][off:off + 64, sl], o_sb[:], rbc[:])

        def po_tiles(h):
            return [po.tile([65, 512], f32, tag="o", name=f"pso{h}_{x}")
                    for x in range(2)]

        def qk_builder(j_tile, pool=None, tag=None):
            """Incremental qk tile construction so its matmuls can be dripped
            into the score pipeline (or the DMA-paced prologue) as filler."""
            pool = pool or pmm
            tag = tag or "mm"
            t = qkp.tile([128, N], bf16, tag="qk", name=f"qk{j_tile}")
            ph = [pool.tile([128, 512], f32, tag=tag, name=f"ps_qk{j_tile}_{x}")
                  for x in range(2)]

            def step(kt):
                for half in range(2):
                    sl = bass.ts(half, 512)
                    nc.tensor.matmul(
                        ph[half][:], wq[kt][:, j_tile * 128:(j_tile + 1) * 128],
                        xt[kt][:, sl],
                        start=(kt == 0), stop=(kt == KT - 1 and not has_bias))
                    if has_bias and kt == KT - 1:
                        nc.tensor.matmul(
                            ph[half][:], wqb[:, j_tile * 128:(j_tile + 1) * 128],
                            xones[:, sl], start=False, stop=True)

            def finish():
                for half in range(2):
                    nc.vector.tensor_copy(t[:, bass.ts(half, 512)], ph[half][:])
                qk[j_tile] = t

            return step, finish

        def v_builder(nt):
            """Incremental v tile so its matmuls can fill pair-0 score holes."""
            dst = vv[nt][:].rearrange("p (h w) -> p h w", w=65)
            phs = [pmm.tile([128, 512], f32, tag="mm", name=f"ps_v{nt}_{x}")
                   for x in range(2)]

            def step(kt):
                for half in range(2):
                    sl = bass.ds(2 * C + half * 512, 512)
                    nc.tensor.matmul(
                        phs[half][:], xt[kt][:, nt * 128:(nt + 1) * 128],
                        wq[kt][:, sl],
                        start=(kt == 0), stop=(kt == KT - 1 and not has_bias))
                    if has_bias and kt == KT - 1:
                        nc.tensor.matmul(
                            phs[half][:], xones[:, nt * 128:(nt + 1) * 128],
                            wqb[:, sl], start=False, stop=True)

            def finish():
                for half in range(2):
                    nc.vector.tensor_copy(
                        dst[:, half * 8:(half + 1) * 8, 0:64],
                        phs[half][:].rearrange("p (h w) -> p h w", w=64))
                nc.gpsimd.memset(dst[:, :, 64:65], 1.0)

            return step, finish

        # ---- stage 1 prologue, kt-major: while the weights stream in, build
        # THREE qk tiles in parallel (6 accumulators across the mm/s/o slots)
        # so each wq[kt] arrival unlocks 6 matmuls instead of 2. Then pair-0
        # scores immediately (starts the ACT exp pipeline early) with v[0]'s
        # matmuls dripped in as PE filler, then the rest of v, then qk9.
        b0s, b0f = qk_builder(0)
        b8s, b8f = qk_builder(8, pool=pss, tag="s")
        b1s, b1f = qk_builder(1, pool=po, tag="o")
        for kt in range(KT):
            b0s(kt)
            b8s(kt)
            b1s(kt)
        b0f()
        b8f()
        b1f()
        v0_step, v0_fin = v_builder(0)
        es0 = []
        for j in range(NT):
            es0.append(scores_j(0, 1, j))
            v0_step(j)
        v0_fin()
        for nt in range(1, NT):
            v_tile(nt, pools=(pmm, pmm))
        qk_tile(9)
        psos = po_tiles(0)
        for j in range(NT):
            pv_step(0, psos, j, es0[j][0])
        norm(0, psos)
        psos = po_tiles(1)
        for j in range(NT):
            pv_step(1, psos, j, es0[j][1])
        norm(1, psos)

        # ---- stage 2 pairs 1..7: software-pipelined per nk-tile j:
        #   scores(j) | PV(h0, j-1) | 2 accumulation steps of the NEXT pair's
        # q-tile (j<4) / k-tile (j>=4). h1's PV closes the pair. The explicit
        # interleave keeps the PE gap-free (a starved PE re-throttles the HAM
        # clock gate to 1.2 GHz, costing far more than the hole itself).
        proj0 = None
        for pair in range(1, 8):
            h0, h1 = 2 * pair, 2 * pair + 1
            filler = []
            if pair < 7:
                q_step, q_fin = qk_builder(pair + 1)
                filler = [(q_step, kt) for kt in range(KT)] + [(q_fin, None)]
            else:
                # Pair 7 has no next qk: drip the first proj tile's kt 0..6
                # (they only need ot[0..6]) into the score holes instead. Uses
                # the pmm slots, which v11's proj used for nt=0 anyway.
                osb0 = outp.tile([128, N], f32, tag="osb", name="osb0")
                php = [pmm.tile([128, 512], f32, tag="mm", name=f"ps_p0_{x}")
                       for x in range(2)]

                def p0_step(kt):
                    for half in range(2):
                        sl = bass.ts(half, 512)
                        nc.tensor.matmul(
                            php[half][:], ot[kt][:, 0:128], wp[kt][:, sl],
                            start=(kt == 0),
                            stop=(kt == KT - 1 and not has_bias))
                        if has_bias and kt == KT - 1:
                            nc.tensor.matmul(
                                php[half][:], xones[:, 0:128], wpb[:, sl],
                                start=False, stop=True)

                proj0 = (p0_step, php, osb0)
                filler = [(p0_step, kt) for kt in range(KT - 1)]
            es = []
            psos0 = po_tiles(h0)
            fi = 0
            for j in range(NT):
                es.append(scores_j(h0, h1, j))
                if j >= 1:
                    pv_step(h0, psos0, j - 1, es[j - 1][0])
                take = 1 if j < NT - 1 else len(filler) - fi
                for _ in range(max(0, take)):
                    if fi < len(filler):
                        fn, arg = filler[fi]
                        fn(arg) if arg is not None else fn()
                        fi += 1
            pv_step(h0, psos0, NT - 1, es[NT - 1][0])
            norm(h0, psos0)
            filler2 = []
            if pair < 7:
                k_step, k_fin = qk_builder(8 + pair + 1)
                filler2 = [(k_step, kt) for kt in range(KT)] + [(k_fin, None)]
            fi = 0
            psos1 = po_tiles(h1)
            if pair == 7:
                # Last head: run the two nq-halves as sequential passes so
                # half0's norm chain (the only fully-exposed one in the whole
                # kernel) starts ~1.7us earlier, hidden under half1's PV.
                for j in range(NT):
                    nc.tensor.matmul(
                        psos1[0][:], vv[j][:, h1 * 65:(h1 + 1) * 65],
                        es[j][1][:, 0:512], start=(j == 0), stop=(j == NT - 1))
                norm(h1, psos1, halves=(0,))
                for j in range(NT):
                    nc.tensor.matmul(
                        psos1[1][:], vv[j][:, h1 * 65:(h1 + 1) * 65],
                        es[j][1][:, 512:1024], start=(j == 0), stop=(j == NT - 1))
                norm(h1, psos1, halves=(1,))
            else:
                for j in range(NT):
                    pv_step(h1, psos1, j, es[j][1])
                    take = 1 if j < NT - 1 else len(filler2) - fi
                    for _ in range(max(0, take)):
                        if fi < len(filler2):
                            fn, arg = filler2[fi]
                            fn(arg) if arg is not None else fn()
                            fi += 1
                norm(h1, psos1)

        # ---- stage 3: proj (orientation B: final[n_tile, co]). nt=0 was
        # prebuilt through kt=6 as pair-7 filler; finish it first.
        p0_step, php, osb0 = proj0
        p0_step(KT - 1)
        for half in range(2):
            nc.vector.tensor_copy(osb0[:, bass.ts(half, 512)], php[half][:])
        nc.sync.dma_start(out_d[0:128, :], osb0[:])
        for nt in range(1, NT):
            osb = outp.tile([128, N], f32, tag="osb", name=f"osb{nt}")
            for half in range(2):
                sl = bass.ts(half, 512)
                # nt=1 rides entirely on the "s" slots: they free right after
                # the last exp, so its kt 0..6 fill the final norm-chain gap
                # (the pmm slots stay held by the nt=0 prefill until then).
                if nt == 1:
                    pool = pss
                else:
                    pool = pss if (2 * nt + half) % 3 == 2 else pmm
                ps = pool.tile([128, 512], f32,
                               tag="mm" if pool is pmm else "s",
                               name=f"ps_p{nt}_{half}")
                for kt in range(KT):
                    nc.tensor.matmul(
                        ps[:], ot[kt][:, nt * 128:(nt + 1) * 128], wp[kt][:, sl],
                        start=(kt == 0), stop=(kt == KT - 1 and not has_bias))
                if has_bias:
                    nc.tensor.matmul(
                        ps[:], xones[:, nt * 128:(nt + 1) * 128], wpb[:, sl],
                        start=False, stop=True)
                nc.vector.tensor_copy(osb[:, sl], ps[:])
            nc.sync.dma_start(out_d[nt * 128:(nt + 1) * 128, :], osb[:])


def _get_compiled(has_bias):
    key = ("nc", has_bias)
    if key in _CACHE:
        return _CACHE[key]
    import concourse.bass as bass
    import concourse.mybir as mybir
    from concourse import bacc, tile

    nc = bacc.Bacc("TRN2", target_bir_lowering=False, debug=False, num_devices=B)
    with tile.TileContext(nc) as tc:
        _build_graph(nc, tc, bass, mybir, has_bias)
    nc.compile()
    _CACHE[key] = nc
    return nc


def _in_maps(x, w_qkv, b_qkv, w_proj, b_proj):
    xT = np.ascontiguousarray(np.transpose(np.asarray(x, np.float32), (0, 2, 1))).astype(BF16)
    ones = np.ones((1, N), BF16)
    wq = np.concatenate([np.asarray(w_qkv, np.float32).T,
                         np.asarray(b_qkv, np.float32)[None, :]], 0).astype(BF16)
    wp = np.concatenate([np.asarray(w_proj, np.float32).T,
                         np.asarray(b_proj, np.float32)[None, :]], 0).astype(BF16)
    wq = np.ascontiguousarray(wq)
    wp = np.ascontiguousarray(wp)
    return [
        {"xT": np.ascontiguousarray(np.concatenate([xT[b], ones], 0)),
         "wqkvT": wq, "wprojT": wp}
        for b in range(B)
    ]


def _ensure_ntff_hook():
    """The agent image's `antenv` lacks `axon_hooks`, so trace=True would
    crash on import. Provide the registry module and install the ctypes
    hook so neuron-profile NTFF capture works. Only used when tracing."""
    import importlib
    import types

    try:
        importlib.import_module("antenv.axon_hooks")
        return
    except ImportError:
        pass
    mod = types.ModuleType("antenv.axon_hooks")
    mod._hook = None

    def set_axon_ntff_profile_hook(h):
        mod._hook = h

    def get_axon_ntff_profile_hook():
        return mod._hook

    mod.set_axon_ntff_profile_hook = set_axon_ntff_profile_hook
    mod.get_axon_ntff_profile_hook = get_axon_ntff_profile_hook
    import antenv

    antenv.axon_hooks = mod
    sys.modules["antenv.axon_hooks"] = mod
    try:
        from trn_agent_boot.trn_boot import _ntff_profile_via_ctypes

        hook = _ntff_profile_via_ctypes("/opt/axon/libaxon_pjrt.so")
        if hook is not None:
            mod._hook = hook
    except Exception:
        pass


def kernel(x, w_qkv, b_qkv, w_proj, b_proj):
    global LAST_RESULTS
    import os

    if os.environ.get("BASS_TRACE"):
        _ensure_ntff_hook()
    from concourse.bass_utils import run_bass_kernel_spmd

    has_bias = bool(np.any(np.asarray(b_qkv)) or np.any(np.asarray(b_proj)))
    nc = _get_compiled(has_bias)
    maps = _in_maps(x, w_qkv, b_qkv, w_proj, b_proj)
    res = run_bass_kernel_spmd(nc, maps, core_ids=list(range(B)))
    LAST_RESULTS = res
    return np.stack([res.results[b]["out"] for b in range(B)]).astype(np.float32)
